# revision 26
# baseline (speedup 1.0000x reference)
"""Causal self-attention Trainium2 kernel (8 NeuronCores, SPMD).

Problem: B=2, T=2048, D=1024, H=16 heads (head_dim 64), fp32 I/O.
    qkv = x @ Wqkv + bqkv ; per-head causal softmax(q k^T / 8) @ v ; out @ Wout + bout

Sharding: 2 batch groups x 4 cores. Core c: batch b=c//4, head group g=c%4
(heads 4g..4g+3, i.e. D-slice [256g, 256g+256)), and out-proj column slice
[256g, 256g+256). Attention outputs are AllGathered (bf16) within each
4-core batch group per query chunk; out-proj is column-sharded so the
final output needs no reduction -- each core returns a [256, 2048] slice
(transposed) which the host reassembles.

Query chunking: 512, 512, 512, 384, 128. The last 512-token chunk is split
384+128 so the FINAL AllGather is only 64 KB and its latency hides under the
128-wide sub-chunk's attention; the kernel tail is norm(128) + small AG +
proj(128) instead of norm(512) + 256 KB AG + proj(512).

Layouts on device (all matmuls bf16 with fp32 PSUM accumulation):
  - x^T [1024, 2048] per batch (host-transposed, bf16)
  - qT/kT [d_local=256, tok] computed directly (W stationary, x^T moving)
  - V [tok, d_local=256] computed directly (x^T tiles stationary, Wv moving)
  - S^T[k, q] = (kT tile).T @ qT  (row-packed pairs of heads, K=64)
  - P = exp(0.125 * S^T) on ACT, no max-subtraction (logits are O(1) by
    construction: weights scaled 0.02), bf16, causal triangle mask applied
    to diagonal 128x128 windows on GpSimd; fully-masked columns never computed
  - PV^T[d, q] = V.T @ P per key-tile, accumulated in PSUM (no transposes)
  - row-sums of P via ones-vector matmuls packed 4-heads/slot (M=32 col tiles)
  - normalize by reciprocal on DVE, folded into the PSUM->SBUF copy

Scheduling notes (why the explicit no-sync deps):
  - ps_s pool is double-buffered (4 PSUM banks) so the S matmuls of the next
    half-tile issue while ACT still reads the previous exp input; without it
    the exp->S->exp chain serializes at ~1.7us per half-tile.
  - Tile's compile-time scheduler places instructions against its own cost
    model; AllGather-gated proj matmuls must be pinned (add_dep_helper,
    sync=False) behind attention landmarks or they get placed early in the
    PE FIFO and head-of-line block attention behind a slow AG.
"""

import numpy as np
import ml_dtypes

import concourse.bass as bass
import concourse.tile as tile
from concourse import bacc, bass_utils, mybir

BF16 = mybir.dt.bfloat16
F32 = mybir.dt.float32

B, T, D, H = 2, 2048, 1024, 16
HD = D // H  # 64
NCORES = 8
GROUPS = [[0, 1, 2, 3], [4, 5, 6, 7]]
P = 128  # partitions
FS = D // P  # 8 feature slices
NTC = T // 512  # 4 key/token chunks (k/v tiling; fixed)
DL = 256  # local d (4 heads * 64)
NMT = DL // P  # 2 stationary M-tiles

# query chunks: (q_lo, q_len); last 512 split 384+128 to shrink the tail
QCHUNKS = [(0, 512), (512, 512), (1024, 512), (1536, 384), (1920, 128)]
NQC = len(QCHUNKS)


def build_bass():
    nc = bacc.Bacc("TRN2", target_bir_lowering=False, debug=False,
                   num_devices=NCORES)

    xt_d = nc.dram_tensor("xt", [D, T], BF16, kind="ExternalInput")
    wq_d = nc.dram_tensor("wq", [D, DL], BF16, kind="ExternalInput")
    wk_d = nc.dram_tensor("wk", [D, DL], BF16, kind="ExternalInput")
    wv_d = nc.dram_tensor("wv", [D, DL], BF16, kind="ExternalInput")
    wo_d = nc.dram_tensor("wout", [D, DL], BF16, kind="ExternalInput")
    bq_d = nc.dram_tensor("bq", [P, NMT], F32, kind="ExternalInput")
    bk_d = nc.dram_tensor("bk", [P, NMT], F32, kind="ExternalInput")
    bv_d = nc.dram_tensor("bv", [P, DL], F32, kind="ExternalInput")
    bo_d = nc.dram_tensor("bo", [P, NMT], F32, kind="ExternalInput")
    tri_d = nc.dram_tensor("tri", [P, P], BF16, kind="ExternalInput")
    ones_d = nc.dram_tensor("ones", [P, 32], BF16, kind="ExternalInput")
    outT_d = nc.dram_tensor("outT", [DL, T], F32, kind="ExternalOutput")

    # chunks 0-1 share ONE AllGather (issued after chunk 1's normalize):
    # fewer collectives on the serial CC stream means the later AGs start
    # on time instead of queueing behind skew-inflated predecessors.
    ag_in = [nc.dram_tensor("ag_in01", [DL, 1024], BF16)] * 2 + [
        nc.dram_tensor(f"ag_in{ci}", [DL, ql], BF16)
        for ci, (_, ql) in list(enumerate(QCHUNKS))[2:]]
    ag_out = [nc.dram_tensor("ag_out01", [D, 1024], BF16)] * 2 + [
        nc.dram_tensor(f"ag_out{ci}", [D, ql], BF16)
        for ci, (_, ql) in list(enumerate(QCHUNKS))[2:]]
    ag_warm_in = nc.dram_tensor("ag_warm_in", [32, 16], BF16)
    ag_warm_out = nc.dram_tensor("ag_warm_out", [128, 16], BF16)

    with tile.TileContext(nc) as tc:
        with (
            tc.tile_pool(name="const", bufs=1) as const,
            tc.tile_pool(name="expst", bufs=3) as expst_pool,
            tc.tile_pool(name="attn", bufs=2) as attn_pool,
            tc.tile_pool(name="agf", bufs=3) as agf_pool,
            tc.tile_pool(name="outsb", bufs=2) as out_pool,
            tc.tile_pool(name="recip", bufs=2) as recip_pool,
            tc.tile_pool(name="ps_s", bufs=2, space="PSUM") as ps_s_pool,
            tc.tile_pool(name="ps_pv", bufs=2, space="PSUM") as ps_pv_pool,
            tc.tile_pool(name="ps_sum", bufs=1, space="PSUM") as ps_sum_pool,
            tc.tile_pool(name="ps_mm", bufs=1, space="PSUM") as ps_mm_pool,
        ):
            # ---- constant loads, ordered by first use -------------------
            xt_view = xt_d[:].rearrange("(s p) t -> p s t", p=P)
            wq_sb = const.tile([P, FS, DL], BF16)
            nc.sync.dma_start(wq_sb[:], wq_d[:].rearrange("(s p) n -> p s n", p=P))
            wk_sb = const.tile([P, FS, DL], BF16)
            nc.sync.dma_start(wk_sb[:], wk_d[:].rearrange("(s p) n -> p s n", p=P))
            xt_tc = [const.tile([P, FS, 512], BF16, tag=f"xt{i}", name=f"xt{i}")
                     for i in range(NTC)]
            for s in range(FS):
                nc.sync.dma_start(xt_tc[0][:, s, :], xt_view[:, s, 0:512])
            bq_sb = const.tile([P, NMT], F32)
            nc.sync.dma_start(bq_sb[:], bq_d[:])
            bk_sb = const.tile([P, NMT], F32)
            nc.sync.dma_start(bk_sb[:], bk_d[:])
            wv_sb = const.tile([P, FS, DL], BF16)
            nc.sync.dma_start(wv_sb[:], wv_d[:].rearrange("(s p) n -> p s n", p=P))
            bv_sb = const.tile([P, DL], F32)
            nc.sync.dma_start(bv_sb[:], bv_d[:])
            tri_sb = const.tile([P, P], BF16)
            nc.sync.dma_start(tri_sb[:], tri_d[:])
            ones_sb = const.tile([P, 32], BF16)
            nc.sync.dma_start(ones_sb[:], ones_d[:])
            zb = const.tile([P, 1], F32)
            nc.gpsimd.memset(zb[:], 0.0)
            for tcidx in range(1, NTC):
                nc.sync.dma_start(xt_tc[tcidx][:],
                                  xt_view[:, :, 512 * tcidx:512 * tcidx + 512])
            wo_sb = const.tile([P, FS, DL], BF16)
            nc.sync.dma_start(wo_sb[:], wo_d[:].rearrange("(s p) n -> p s n", p=P))
            bo_sb = const.tile([P, NMT], F32)
            nc.sync.dma_start(bo_sb[:], bo_d[:])

            qT_tc = [const.tile([P, NMT, 512], BF16, tag=f"qT{i}", name=f"qT{i}") for i in range(NTC)]
            kT_tc = [const.tile([P, NMT, 512], BF16, tag=f"kT{i}", name=f"kT{i}") for i in range(NTC)]
            v_tc = [const.tile([P, 4, DL], BF16, tag=f"v{i}", name=f"v{i}") for i in range(NTC)]

            def qkv_groups(tcx, pool=None, vpool=None):
                """q/k + v matmul groups for one 512-token chunk, returned
                as closures so they can be interleaved into the attention
                stream (fills PE idle while ACT runs exp)."""
                xt = xt_tc[tcx]
                mmtile = (lambda: ps_mm_pool.tile([P, 512], F32, tag="mm",
                                                  name="mmps")) \
                    if pool is None else pool
                vtile = (lambda: ps_mm_pool.tile([P, 512], F32, tag="mm",
                                                 name="mmps")) \
                    if vpool is None else vpool

                def qk_group(dst, w_sb, b_sb, mt):
                    def emit(after=None):
                        ps = mmtile()
                        for s in range(FS):
                            mm = nc.tensor.matmul(
                                ps[:], w_sb[:, s, P * mt:P * mt + P],
                                xt[:, s, :],
                                start=(s == 0), stop=(s == FS - 1))
                            if after is not None and s == 0:
                                tile.add_dep_helper(mm.ins, after, sync=False,
                                                    reason="filler order")
                        nc.vector.tensor_scalar_add(
                            dst[:, mt, :], ps[:], b_sb[:, mt:mt + 1])
                    return emit

                def v_group(tt):
                    # consecutive v groups use alternating halves of the
                    # psum bank so tt+1's matmuls don't wait tt's epilogue
                    lo = DL * (tt % 2)

                    def emit(after=None):
                        ps = vtile()
                        for s in range(FS):
                            mm = nc.tensor.matmul(
                                ps[:, lo:lo + DL], xt[:, s, P * tt:P * tt + P],
                                wv_sb[:, s, :],
                                start=(s == 0), stop=(s == FS - 1))
                            if after is not None and s == 0:
                                tile.add_dep_helper(mm.ins, after, sync=False,
                                                    reason="filler order")
                        nc.vector.tensor_add(
                            v_tc[tcx][:, tt, :], ps[:, lo:lo + DL], bv_sb[:])
                    return emit

                gs = []
                for dst, w_sb, b_sb in ((qT_tc[tcx], wq_sb, bq_sb),
                                        (kT_tc[tcx], wk_sb, bk_sb)):
                    for mt in range(NMT):
                        gs.append(qk_group(dst, w_sb, b_sb, mt))
                vs = [v_group(tt) for tt in range(4)]
                return gs, vs

            def attention_chunk(ci, fillers=(), pin_fillers=False):
                """Attention for query chunk ci; returns the last PV matmul
                instruction (for pinning tail work behind it)."""
                q_lo, qlen = QCHUNKS[ci]
                q_tck, q_off0 = q_lo // 512, q_lo % 512
                nkk = (q_lo + qlen) // P
                fillers = list(fillers)
                fill_every = max(1, nkk // max(1, len(fillers))) if fillers else 0
                ps_pv = [ps_pv_pool.tile([P, 512], F32, tag="pv",
                                         name=f"pv{ci}_{i}") for i in range(2)]
                ps_sum = ps_sum_pool.tile([P, 512], F32)
                last_mm = [None]

                def geom(kk):
                    tck, m = kk // 4, kk % 4
                    d = P * kk - q_lo
                    if d < 0:
                        return tck, m, False, 0, qlen
                    return tck, m, True, d, qlen - d

                # narrow chunks (q_len <= 128): all 4 heads' S fit in ONE
                # psum bank -> one exp instruction per key tile instead of
                # two (saves the 352-cycle ACT issue overhead per kk).
                narrow = False

                def eslice(expst, h, n):
                    if narrow:
                        return expst[:, 0, qlen * h:qlen * h + n]
                    return expst[:, h, 0:n]

                def emit_s_exp(kk):
                    """S^T in two 2-head halves, each its own 2-bank psum
                    tile (pool bufs=2) so S(g+1) issues while ACT still
                    reads exp(g)'s input."""
                    tck, m, diag, off, W = geom(kk)
                    expst = expst_pool.tile([P, 4, 512], BF16, tag="expst",
                                            name=f"expst{ci}_{kk}")
                    if narrow:
                        ps_s = ps_s_pool.tile([P, 2, 512], F32, tag="s",
                                              name=f"s{ci}_{kk}")
                        for h in range(4):
                            mt, rp = h // 2, 64 * (h % 2)
                            mm = nc.tensor.matmul(
                                ps_s[:, 0, qlen * h:qlen * h + W],
                                kT_tc[tck][rp:rp + 64, mt, P * m:P * m + P],
                                qT_tc[q_tck][rp:rp + 64, mt,
                                             q_off0 + off:q_off0 + off + W],
                                start=True, stop=True)
                            last_mm[0] = mm.ins
                        nc.scalar.activation(
                            expst[:, 0, 0:4 * qlen],
                            ps_s[:, 0, 0:4 * qlen],
                            mybir.ActivationFunctionType.Exp,
                            bias=zb[:], scale=0.125)
                    else:
                        for hp in range(2):
                            ps_s = ps_s_pool.tile([P, 2, 512], F32, tag="s",
                                                  name=f"s{ci}_{kk}_{hp}")
                            for hh in range(2):
                                h = 2 * hp + hh
                                mt, rp = h // 2, 64 * (h % 2)
                                mm = nc.tensor.matmul(
                                    ps_s[:, hh, 0:W],
                                    kT_tc[tck][rp:rp + 64, mt,
                                               P * m:P * m + P],
                                    qT_tc[q_tck][rp:rp + 64, mt,
                                                 q_off0 + off:q_off0 + off + W],
                                    start=True, stop=True)
                                last_mm[0] = mm.ins
                            nc.scalar.activation(
                                expst[:, 2 * hp:2 * hp + 2, 0:W],
                                ps_s[:, 0:2, 0:W],
                                mybir.ActivationFunctionType.Exp,
                                bias=zb[:], scale=0.125)
                    if diag:
                        # SBUF-only bf16 muls -> idle GpSimd, keeping DVE
                        # free for the psum-slot-releasing epilogues
                        for h in range(4):
                            nc.gpsimd.tensor_mul(
                                eslice(expst, h, P),
                                eslice(expst, h, P), tri_sb[:])
                    return expst

                def emit_pv_sums(kk, expst):
                    tck, m, diag, off, W = geom(kk)
                    # PV^T accumulation (V stationary, exp moving), 2 heads/slot
                    for hp in range(2):
                        for hh in range(2):
                            h = 2 * hp + hh
                            mm = nc.tensor.matmul(
                                ps_pv[hp][64 * hh:64 * hh + 64, off:off + W],
                                v_tc[tck][:, m, 64 * h:64 * h + 64],
                                eslice(expst, h, W),
                                start=(kk == 0), stop=(kk == nkk - 1))
                            last_mm[0] = mm.ins
                    # softmax denominators: ones-matmuls (M=32 so each head's
                    # sum lands replicated on 32 partitions), 4 heads packed
                    # by 32-aligned column groups (partitions 32h..32h+31)
                    for h in range(4):
                        nc.tensor.matmul(
                            ps_sum[32 * h:32 * h + 32, off:off + W],
                            ones_sb[:, 0:32],
                            eslice(expst, h, W),
                            start=(kk == 0), stop=(kk == nkk - 1),
                            tile_position=(0, 32 * h))

                def pop_filler(f):
                    f(last_mm[0] if pin_fillers else None)

                # Software-pipelined emission: S+exp for kk+1 go into the
                # engine queues BEFORE PV/sums for kk, so the next exp's
                # inputs are computed while ACT processes the current one --
                # neither engine waits on the serial S->exp->PV chain.
                expst_prev = emit_s_exp(0)
                for kk in range(1, nkk):
                    expst_cur = emit_s_exp(kk)
                    emit_pv_sums(kk - 1, expst_prev)
                    expst_prev = expst_cur
                    if fillers and kk % fill_every == 0:
                        pop_filler(fillers.pop(0))
                emit_pv_sums(nkk - 1, expst_prev)
                last_pv = last_mm[0]
                while fillers:
                    pop_filler(fillers.pop(0))
                # normalize + stage for the AllGather. Chain the muls with
                # no-sync deps so hp0 finishes (and releases its PV psum
                # slot for the next chunk) before hp1 starts.
                recip = recip_pool.tile([P, 512], F32)
                nc.vector.reciprocal_approx_fast(recip[:, 0:qlen],
                                                 ps_sum[:, 0:qlen])
                # hp0-first chaining releases ps_pv[0] for the next chunk's
                # PV; pointless (and latency-adding) for the final chunk
                chain = True
                prev_mul = None
                for hp in range(2):
                    attn = attn_pool.tile([P, 512], BF16)
                    for hh in range(2):
                        h = 2 * hp + hh
                        for half in range(2):
                            lo = 64 * hh + 32 * half
                            mul = nc.vector.tensor_mul(
                                attn[lo:lo + 32, 0:qlen],
                                ps_pv[hp][lo:lo + 32, 0:qlen],
                                recip[32 * h:32 * h + 32, 0:qlen])
                            if chain and prev_mul is not None:
                                tile.add_dep_helper(
                                    mul.ins, prev_mul.ins, sync=False,
                                    reason="normalize order hp0-first")
                            prev_mul = mul
                    nc.sync.dma_start(
                        ag_in[ci][P * hp:P * hp + P,
                                  q_lo - QCHUNKS[AG_HEAD[ci]][0]:
                                  q_lo - QCHUNKS[AG_HEAD[ci]][0] + qlen],
                        attn[:, 0:qlen])
                if ci == AG_TAIL[ci]:
                    ag_chunk(ci)
                return last_pv

            # AG groups: chunks 0-1 share one collective (head chunk 0,
            # triggered after tail chunk 1); 2, 3, 4 are their own.
            AG_HEAD = [0, 0, 2, 3, 4]
            AG_TAIL = [1, 1, 2, 3, 4]
            agf_tiles = {}

            def ag_chunk(ci):
                glen = sum(QCHUNKS[c][1] for c in range(NQC)
                           if AG_HEAD[c] == AG_HEAD[ci])
                nc.gpsimd.collective_compute(
                    "AllGather", mybir.AluOpType.bypass,
                    replica_groups=GROUPS,
                    ins=[ag_in[ci][:]], outs=[ag_out[ci][:]])
                if ci <= 1:  # one-shot big tile; const pool (bufs=1)
                    agf = const.tile([P, FS, 1024], BF16, tag="agf01",
                                     name="agf01")
                else:
                    agf = agf_pool.tile([P, FS, 512], BF16, name=f"agf{ci}")
                # two half DMAs: proj can start on the first half while the
                # second lands, without paying 8 separate DMA-issue costs
                agv = ag_out[ci][:].rearrange("(s p) t -> p s t", p=P)
                nc.sync.dma_start(agf[:, 0:4, 0:glen], agv[:, 0:4, :])
                nc.sync.dma_start(agf[:, 4:8, 0:glen], agv[:, 4:8, :])
                agf_tiles[AG_HEAD[ci]] = agf

            def proj_groups(ci, scalar_epilogue=False):
                q_lo, qlen = QCHUNKS[ci]
                agoff = q_lo - QCHUNKS[AG_HEAD[ci]][0]

                def group(mt):
                    # for narrow chunks the two m-tiles fit in disjoint
                    # halves of the psum bank -> no serialization via the
                    # mt0 epilogue
                    lo = 0

                    def emit(after=None):
                        agf = agf_tiles[AG_HEAD[ci]]
                        ps = ps_mm_pool.tile([P, 512], F32, tag="mm")
                        for s in range(FS):
                            mm = nc.tensor.matmul(
                                ps[:, lo:lo + qlen],
                                wo_sb[:, s, P * mt:P * mt + P],
                                agf[:, s, agoff:agoff + qlen],
                                start=(s == 0), stop=(s == FS - 1))
                            if after is not None and s == 0:
                                tile.add_dep_helper(mm.ins, after, sync=False,
                                                    reason="proj after attn")
                        osb = out_pool.tile([P, 512], F32)
                        if scalar_epilogue:
                            # tail projs: ACT is idle there, DVE is not
                            nc.scalar.add(osb[:, 0:qlen], ps[:, lo:lo + qlen],
                                          bo_sb[:, mt:mt + 1])
                        else:
                            nc.vector.tensor_scalar_add(
                                osb[:, 0:qlen], ps[:, lo:lo + qlen],
                                bo_sb[:, mt:mt + 1])
                        nc.sync.dma_start(
                            outT_d[P * mt:P * mt + P, q_lo:q_lo + qlen],
                            osb[:, 0:qlen])
                    return emit
                return [group(mt) for mt in range(NMT)]

            # Emission order shapes the psum-slot queues and Tile's
            # cumulative per-engine sync counters. qkv(c+1) matmul groups are
            # interleaved INTO attention(c)'s grid-point stream (fills PE
            # while ACT runs exp); out-proj groups are pinned behind
            # attention landmarks so nothing AllGather-gated ever sits ahead
            # of attention work in the PE FIFO.
            # warm up the PE clock (HAM) with throwaway matmuls while the
            # input DMAs land, so qkv(0) runs at 2.4 GHz from the start
            warm_sb = const.tile([P, 512], BF16)
            nc.gpsimd.memset(warm_sb[:], 0.0)
            ps_w = ps_mm_pool.tile([P, 512], F32, tag="mm")
            for _ in range(32):
                nc.tensor.matmul(ps_w[:], warm_sb[:, 0:P], warm_sb[:],
                                 start=True, stop=True)
            # dummy collective: pay the cc entry-barrier during the DMA
            # window instead of on the first real AllGather
            nc.gpsimd.collective_compute(
                "AllGather", mybir.AluOpType.bypass, replica_groups=GROUPS,
                ins=[ag_warm_in[:]], outs=[ag_warm_out[:]])

            # q/k(0) double-buffer through the ps_s slots and v(0) through
            # the (still idle) ps_pv slots -- two independent psum chains
            # run concurrently before attention(0) starts.
            qk0, v0 = qkv_groups(
                0,
                pool=lambda: ps_s_pool.tile(
                    [P, 2, 512], F32, tag="s", name="qkv0mm")[:, 0, :],
                vpool=lambda: ps_pv_pool.tile(
                    [P, 512], F32, tag="pv", name="qkv0v"))
            for g in qk0:
                g()
            for g in v0:
                g()
            # qk(1) feeds attention(1)'s S stream soon after chunk 0, so it
            # fills chunk 0 (q groups first); v/qk of later chunks spread
            # through the wider chunks.
            qk1, v1 = qkv_groups(1)
            attention_chunk(0, qk1)
            qk2, v2 = qkv_groups(2)
            attention_chunk(1, v1 + qk2)
            qk3, v3 = qkv_groups(3)
            attention_chunk(2, v2 + qk3)
            # proj(0) is gated on the merged AG(0-1): pin it late in c3a
            # (the AG completes mid-c3a); proj(1)/proj(2) go into the final
            # 128-wide chunk; proj(3a)/proj(3b) run in the tail.
            attention_chunk(3, v3 + proj_groups(0), pin_fillers=True)
            last_pv = attention_chunk(
                4, proj_groups(1) + proj_groups(2), pin_fillers=True)
            for g in proj_groups(3, scalar_epilogue=True):
                g(last_pv)
            for g in proj_groups(4, scalar_epilogue=True):
                g(last_pv)

    nc.compile()
    return nc


_NC_CACHE = None


def _get_nc():
    global _NC_CACHE
    if _NC_CACHE is None:
        _NC_CACHE = build_bass()
    return _NC_CACHE


def _make_in_maps(x, Wqkv, bqkv, Wout, bout):
    bf16 = ml_dtypes.bfloat16
    in_maps = []
    for c in range(NCORES):
        b, g = c // 4, c % 4
        cs = DL * g  # column/dim slice start for this core's heads
        im = {
            "xt": np.ascontiguousarray(x[b].T).astype(bf16),
            "wq": np.ascontiguousarray(Wqkv[:, cs:cs + DL]).astype(bf16),
            "wk": np.ascontiguousarray(Wqkv[:, D + cs:D + cs + DL]).astype(bf16),
            "wv": np.ascontiguousarray(Wqkv[:, 2 * D + cs:2 * D + cs + DL]).astype(bf16),
            "wout": np.ascontiguousarray(Wout[:, cs:cs + DL]).astype(bf16),
            "bq": np.ascontiguousarray(
                bqkv[cs:cs + DL].reshape(NMT, P).T).astype(np.float32),
            "bk": np.ascontiguousarray(
                bqkv[D + cs:D + cs + DL].reshape(NMT, P).T).astype(np.float32),
            "bv": np.ascontiguousarray(np.broadcast_to(
                bqkv[2 * D + cs:2 * D + cs + DL].reshape(1, DL),
                (P, DL))).astype(np.float32),
            "bo": np.ascontiguousarray(
                bout[cs:cs + DL].reshape(NMT, P).T).astype(np.float32),
            "tri": np.triu(np.ones((P, P))).astype(bf16),
            "ones": np.ones((P, 32), dtype=bf16),
        }
        in_maps.append(im)
    return in_maps


def _run(inputs, trace=False, tmpdir=None):
    nc = _get_nc()
    in_maps = _make_in_maps(**inputs)
    res = bass_utils.run_bass_kernel_spmd(
        nc, in_maps, core_ids=list(range(NCORES)), trace=trace, tmpdir=tmpdir)
    out = np.empty((B, T, D), dtype=np.float32)
    for c in range(NCORES):
        b, g = c // 4, c % 4
        out[b, :, DL * g:DL * g + DL] = res.results[c]["outT"].T
    return out, res


def kernel(x, Wqkv, bqkv, Wout, bout):
    out, _ = _run(dict(x=np.asarray(x, dtype=np.float32),
                       Wqkv=np.asarray(Wqkv, dtype=np.float32),
                       bqkv=np.asarray(bqkv, dtype=np.float32),
                       Wout=np.asarray(Wout, dtype=np.float32),
                       bout=np.asarray(bout, dtype=np.float32)))
    return out
